# revision 58
# baseline (speedup 1.0000x reference)
"""Trainium2 Bass kernel for nn_CAFF_3100966388292.

Dual-stream (SAR/OPT) cross-attention fusion net:
  theta/phi/g 1x1-conv projections on both streams, per-sample NxN attention
  maps fused elementwise, both value streams attended, product taken, output
  1x1-conv + residual + channel-mean pool + linear head.

Strategy (pure data parallel, 4 samples per core on 8 cores):
  * All big matmuls in fp8 DoubleRow (theta/phi/g projections, logits,
    attention-apply). Inputs are host-quantized to e4m3 with error diffusion
    along the channel dim, which keeps channel-contractions and the residual
    colsum (ridden as a ones-column on the g weights) near-bf16 accurate.
  * Attention logits computed TRANSPOSED: L^T(m,n) = phi^T theta, so the
    contracted dim (m) of att@g lands on PSUM partitions naturally.
  * S = Ex*Ey stored e5m2 (global EXP_SHIFT=-8 per stream keeps all row
    maxima inside e5m2 range; e4m3 overflows); the attention-apply contracts
    S against e4m3 g tiles in DoubleRow, halving the baseline's biggest
    bf16 matmul.
  * Softmax denominators: one level of bf16 pair-adds over the six E chunks
    (on GpSimd, consumed a full phase later), then short ones-column
    matmuls; the (Zx*Zy)^-2 fixup runs COLUMNAR on [128,6] tiles after PE
    transposes of the row operands (the serial [1,768] DVE reciprocal of
    the baseline was ~5us per sample). The Zy row and the qraw matvec row
    share one PSUM tile at partition bases 0/64, so a single [65,768] ACT
    copy extracts both (ACT cost depends only on free size); the
    partition-64 transpose uses a partition-replicated identity +
    explicit tile_position.
  * The final W-projection + residual + channel-mean + head collapse
    algebraically:
      pooled(n) = R2(n)*qraw(n) + (ga/C)*sum(W_b) + rs(n),
      rs(n)     = (go/C)*colsum(opt)(n) + (gs/C)*colsum(sar)(n)
      qraw(n)   = sum_ci wbar(ci) * Ux(ci,n) * Uy(ci,n),
      wbar      = (ga/C) * W_w.sum(0)
    which removes the (C,CI)x(CI,N) W matmul entirely.
  * PSUM: a [128,768] "med" pool (bufs=3) + a 1-bank "small" pool (bufs=2)
    for the g-projections = 8 banks.
  * Samples are processed in PAIRS with all cross-engine chains deferred by
    one phase: each pair opens with BOTH samples' g-projections (small
    PSUM pool, no deps on the previous pair) which absorbs the previous
    pair's exp drain and keeps the PE HAM-warm; the previous pair's
    attention-applies and fixup chains interleave between this pair's
    theta/phi blocks; each sample's logits emit as early as possible.
    Engine FIFO order == emission order, so every deferred chain is placed
    where its dependencies are already drained (measured, not guessed:
    several "obvious" interleavings lose 10-25us to head-of-line blocking).
"""

import sys
import types

import ml_dtypes
import numpy as np

# The agent image's antenv package lacks axon_hooks; register the equivalent
# NTFF hook so run_bass_kernel_spmd(trace=True) works if ever requested.
try:  # pragma: no cover
    import antenv.axon_hooks  # noqa: F401
except ImportError:
    try:
        from trn_agent_boot.trn_boot import _ntff_profile_via_ctypes

        _hook = _ntff_profile_via_ctypes("/opt/axon/libaxon_pjrt.so")
        _mod = types.ModuleType("antenv.axon_hooks")
        _mod.get_axon_ntff_profile_hook = lambda: _hook
        _mod.set_axon_ntff_profile_hook = lambda h: None
        sys.modules["antenv.axon_hooks"] = _mod
    except Exception:
        pass

import concourse.bass as bass
import concourse.tile as tile
from concourse import bacc, mybir
from concourse.bass_utils import run_bass_kernel_spmd

F32 = mybir.dt.float32
BF16 = mybir.dt.bfloat16
FP8 = mybir.dt.float8e4
FP8E5 = mybir.dt.float8e5
EXP_SHIFT = -8.0  # per-stream logit shift; cancels exactly, keeps S in e5m2

B, C, CI, N, HOUT = 32, 512, 256, 768, 256
NCORES = 8
BPC = B // NCORES  # samples per core
KC = C // 128  # 4 k-chunks over channels
MC = N // 128  # 6 chunks over positions
CIC = CI // 128  # 2 chunks over inner channels
GW = 272  # fp8 g-weight free-dim padding (257 cols, DR needs step%16==0)
# matmul windows over N: DoubleRow streams at most 1024 moving elements
# (512 output cols), and a 512-col fp32 window fills one 2KB PSUM bank
NH = ((0, 512), (512, 256))

_cached = {}


def _dither8(a, axis):
    """e4m3 quantization with error diffusion along `axis` (preserves sums)."""
    a = np.moveaxis(np.asarray(a, dtype=np.float32), axis, 0)
    out = np.empty(a.shape, dtype=ml_dtypes.float8_e4m3fn)
    carry = np.zeros(a.shape[1:], np.float32)
    for i in range(a.shape[0]):
        v = a[i] + carry
        q = v.astype(ml_dtypes.float8_e4m3fn)
        carry = v - q.astype(np.float32)
        out[i] = q
    return np.moveaxis(out, 0, axis)


def _pack8(a):
    """(R, F) host array -> (128, R//128, F) partition-major, dithered e4m3."""
    a = np.asarray(a, dtype=np.float32)
    r, f = a.shape
    k = r // 128
    d = _dither8(a, 0)
    return np.ascontiguousarray(d.reshape(k, 128, f).transpose(1, 0, 2))


def _packbf(a):
    """(R, F) host array -> (128, R//128 * F) partition-major bf16."""
    a = np.asarray(a, dtype=np.float32)
    r, f = a.shape
    k = r // 128
    return np.ascontiguousarray(
        a.reshape(k, 128, f).transpose(1, 0, 2).reshape(128, k * f)
    ).astype(ml_dtypes.bfloat16)


def _build(has_gb_x, has_gb_y, has_hb, has_tpb, gs_sc, go_sc):
    nc = bacc.Bacc("TRN2", target_bir_lowering=False, debug=False)
    AF = mybir.ActivationFunctionType
    ALU = mybir.AluOpType

    def mm(out, lhsT, rhs, start, stop):
        nc.tensor.matmul(out, lhsT, rhs, start=start, stop=stop)

    def mmdr(out, lhsT, rhs, start, stop):
        nc.tensor.matmul(out, lhsT, rhs, start=start, stop=stop,
                         perf_mode=mybir.MatmulPerfMode.DoubleRow)

    # inputs host-packed to (BPC, 128, KC*N) partition-major dithered e4m3
    d_sar8 = nc.dram_tensor("sar8", [BPC, 128, KC * N], FP8, kind="ExternalInput")
    d_opt8 = nc.dram_tensor("opt8", [BPC, 128, KC * N], FP8, kind="ExternalInput")
    # host-pretransposed + packed projection weights
    d_w = {}
    for nm in ("wt_tx", "wt_px", "wt_ty", "wt_py"):
        d_w[nm] = nc.dram_tensor(nm, [128, KC * CI], FP8, kind="ExternalInput")
    for nm in ("wt_gx", "wt_gy"):  # g weights carry a ones column at 256
        d_w[nm] = nc.dram_tensor(nm, [128, KC * GW], FP8, kind="ExternalInput")
    d_hwT = nc.dram_tensor("hwT", [128, MC * HOUT], BF16, kind="ExternalInput")
    d_wbar = nc.dram_tensor("wbar", [CI], BF16, kind="ExternalInput")
    d_tb = {}
    if has_tpb:  # theta/phi per-partition bias columns (CI,), fp32 (ACT bias)
        for nm in ("b_tx", "b_px", "b_ty", "b_py"):
            d_tb[nm] = nc.dram_tensor(nm, [CI], F32, kind="ExternalInput")
    d_ones = nc.dram_tensor("ones_col", [128, 1], BF16, kind="ExternalInput")
    need_onesr = has_gb_x or has_gb_y or has_hb
    if need_onesr:
        d_onesr = nc.dram_tensor("ones_row", [1, 128], BF16, kind="ExternalInput")
    d_ident = nc.dram_tensor("ident", [4, 4], F32, kind="ExternalInput")
    d_identr = nc.dram_tensor("identr", [128, 1], F32, kind="ExternalInput")
    d_expb = nc.dram_tensor("expb", [128, 1], F32, kind="ExternalInput")
    d_gb = {}
    if has_gb_x:
        d_gb["x"] = nc.dram_tensor("gb_x", [1, CI], BF16, kind="ExternalInput")
    if has_gb_y:
        d_gb["y"] = nc.dram_tensor("gb_y", [1, CI], BF16, kind="ExternalInput")
    if has_hb:
        d_hb = nc.dram_tensor("hb", [1, HOUT], BF16, kind="ExternalInput")
    d_out = nc.dram_tensor("out", [BPC, HOUT], F32, kind="ExternalOutput")

    with tile.TileContext(nc) as tc, \
            tc.tile_pool(name="wts", bufs=1) as wts, \
            tc.tile_pool(name="inp", bufs=2) as inp, \
            tc.tile_pool(name="proj", bufs=1) as proj, \
            tc.tile_pool(name="att", bufs=1) as attp, \
            tc.tile_pool(name="rows", bufs=1) as rows, \
            tc.tile_pool(name="rtmp", bufs=4) as rtmp, \
            tc.tile_pool(name="psP", bufs=3, space="PSUM") as psP, \
            tc.tile_pool(name="psS", bufs=2, space="PSUM") as psS:

        def med():
            return psP.tile([128, N], F32, tag="ps", name="ps")

        def small():
            return psS.tile([128, 512], F32, tag="small", name="small")

        # ---- DMAs in strict first-use order: the queues are FIFO, so
        # everything emitted ahead of the first matmul's dependencies delays
        # kernel start ----
        def load_w(nm, cols=CI):
            t = wts.tile([128, KC, cols], FP8, tag=nm, name=nm)
            nc.sync.dma_start(t[:], d_w[nm].ap().rearrange("p (k f) -> p k f", k=KC))
            return t

        w_sb = {}
        # the first matmul is the sample-0 g-projection: its weight chunks
        # and the first input chunks gate the whole kernel, so issue those
        # pieces first
        t = wts.tile([128, KC, GW], FP8, tag="wt_gx", name="wt_gx")
        w_sb["wt_gx"] = t
        nc.sync.dma_start(t[:, 0:2, :],
                          d_w["wt_gx"].ap()[:, :2 * GW].rearrange(
                              "p (k f) -> p k f", k=2))
        x8_0 = inp.tile([128, KC, N], FP8, tag="x80", name="x8")
        nc.sync.dma_start(x8_0[:, 0:2, :],
                          d_sar8[0][:, :2 * N].rearrange("p (k n) -> p k n", k=2))
        nc.sync.dma_start(
            t[:, 2:, :],
            d_w["wt_gx"].ap()[:, 2 * GW:].rearrange("p (k f) -> p k f", k=KC - 2))
        nc.sync.dma_start(x8_0[:, 2:, :],
                          d_sar8[0][:, 2 * N:].rearrange("p (k n) -> p k n", k=2))
        w_sb["wt_gy"] = load_w("wt_gy", GW)
        y8_0 = inp.tile([128, KC, N], FP8, tag="y80", name="y8")
        nc.sync.dma_start(y8_0[:], d_opt8[0].rearrange("p (k n) -> p k n", k=KC))
        w_sb["wt_tx"] = load_w("wt_tx")
        w_sb["wt_px"] = load_w("wt_px")
        tb_sb = {}
        if has_tpb:
            for nm, d in d_tb.items():
                t = wts.tile([128, CIC], F32, tag=nm, name=nm)
                nc.sync.dma_start(t[:], d.ap().rearrange("(k p) -> p k", p=128))
                tb_sb[nm] = t
        w_sb["wt_ty"] = load_w("wt_ty")
        w_sb["wt_py"] = load_w("wt_py")

        def load_inputs(s):
            j = s % 2
            x8 = inp.tile([128, KC, N], FP8, tag=f"x8{j}", name="x8")
            y8 = inp.tile([128, KC, N], FP8, tag=f"y8{j}", name="y8")
            nc.sync.dma_start(x8[:], d_sar8[s].rearrange("p (k n) -> p k n", k=KC))
            nc.sync.dma_start(y8[:], d_opt8[s].rearrange("p (k n) -> p k n", k=KC))
            return x8, y8

        in_tiles = [(x8_0, y8_0)]
        in_tiles.append(load_inputs(1))

        # ---- small constants (all needed later than the projections) ----
        wbar = wts.tile([128, CIC], BF16, tag="wbar", name="wbar")
        nc.sync.dma_start(wbar[:], d_wbar.ap().rearrange("(k p) -> p k", p=128))
        ones_col = wts.tile([128, 1], BF16, tag="ones_col", name="ones_col")
        nc.sync.dma_start(ones_col[:], d_ones.ap())
        ident = wts.tile([4, 4], F32, tag="ident", name="ident")
        nc.sync.dma_start(ident[:], d_ident.ap())
        identr = wts.tile([128, 1], F32, tag="identr", name="identr")
        nc.sync.dma_start(identr[:], d_identr.ap())
        expb = wts.tile([128, 1], F32, tag="expb", name="expb")
        nc.sync.dma_start(expb[:], d_expb.ap())
        hwT = wts.tile([128, MC, HOUT], BF16, tag="hwT", name="hwT")
        nc.sync.dma_start(hwT[:], d_hwT.ap().rearrange("p (k f) -> p k f", k=MC))
        if need_onesr:
            ones_row = wts.tile([1, 128], BF16, tag="ones_row", name="ones_row")
            nc.sync.dma_start(ones_row[:], d_onesr.ap())
        gb_sb = {}
        for st, d in d_gb.items():
            t = wts.tile([1, CI], BF16, tag=f"gb_{st}", name=f"gb_{st}")
            nc.sync.dma_start(t[:], d.ap())
            gb_sb[st] = t
        if has_hb:
            hb = wts.tile([1, HOUT], BF16, tag="hb", name="hb")
            nc.sync.dma_start(hb[:], d_hb.ap())

        pooledT = rows.tile([128, BPC, MC], BF16, tag="pooledT", name="pooledT")

        def emit_fixup_z(fx):
            """Zx into PSUM tile A (partition 0); Zy into tile B partition
            0. The qraw matvec later lands in tile B at partition 64, so ONE
            ACT copy extracts zy+q together: ACT cost depends only on the
            free size, so a [65,768] copy costs the same as [1,768]."""
            s, yv, Ep, rscol = fx
            za_ps = med()
            zb_ps = med()
            for sti, zps in enumerate((za_ps, zb_ps)):
                for o, f in NH:
                    for i in range(3):
                        mm(zps[:1, o:o + f], ones_col[:],
                           Ep[:, sti, i, o:o + f], i == 0, i == 2)
            return (s, yv, zb_ps, za_ps, rscol)

        def emit_fixup_q(fx):
            s, yv, zb_ps, za_ps, rscol = fx
            for cic in range(CIC):
                for o, f in NH:
                    mm(zb_ps[64:65, o:o + f], wbar[:, cic:cic + 1],
                       yv[:, cic, o:o + f], cic == 0, cic == CIC - 1)
            zqq = rtmp.tile([65, N], F32, tag="r_zq", name="zqq", bufs=2)
            nc.scalar.copy(zqq[:], zb_ps[0:65, :])
            # zx never leaves PSUM: the zp product reads it directly (one
            # PSUM operand is allowed on DVE), saving an ACT row copy
            zp = rtmp.tile([1, N], F32, tag="r_zp", name="zp", bufs=2)
            nc.vector.tensor_mul(zp[:], zqq[0:1, :], za_ps[:1, :])
            return (s, zp, zqq, rscol)

        def emit_fixup_a(fx):
            return emit_fixup_q(emit_fixup_z(fx))

        def emit_fixup_b(fx):
            """12 PE transposes + columnar reciprocal chain (PE + DVE)."""
            s, zp, zqq, rscol = fx
            tr = small()
            for j in range(MC):
                nc.tensor.transpose(tr[:, j:j + 1],
                                    zp[:1, j * 128:(j + 1) * 128], ident[:1, :1])
            for j in range(MC):
                nc.tensor.transpose(tr[:, 8 + j:9 + j],
                                    zqq[64:65, j * 128:(j + 1) * 128],
                                    identr[64:65, :1], tile_position=(64, 0))
            r1 = rtmp.tile([128, MC], F32, tag="r_c1", name="r1", bufs=2)
            nc.vector.reciprocal(r1[:], tr[:, 0:MC])
            r2 = rtmp.tile([128, MC], F32, tag="r_c2", name="r2", bufs=2)
            nc.vector.tensor_mul(r2[:], r1[:], r1[:])
            p4 = rtmp.tile([128, MC], F32, tag="r_c3", name="p4", bufs=2)
            nc.vector.tensor_mul(p4[:], r2[:], tr[:, 8:8 + MC])
            nc.vector.tensor_add(pooledT[:, s, :], p4[:], rscol[:])

        def emit_apply(ap):
            """Attention-apply of sample s, emitted one projection phase
            later so the exp tail of sample s overlaps sample s+1's
            projection matmuls instead of stalling the PE."""
            s, gT, S, Ep, rscol = ap
            yv = attp.tile([128, CIC, N], BF16, tag=f"yv{s % 2}", name="yv",
                           bufs=2)
            for cic in range(CIC):
                ptu = {}
                for st in ("x", "y"):
                    pt = med()
                    ptu[st] = pt
                    for o, f in NH:
                        for jp in range(MC // 2):
                            mmdr(pt[:, o:o + f],
                                 gT[st][:, 2 * jp:2 * jp + 2,
                                        cic * 128:(cic + 1) * 128],
                                 S[:, 2 * jp:2 * jp + 2, o:o + f],
                                 jp == 0, jp == MC // 2 - 1)
                # DVE tensor_tensor cannot read two PSUM operands; bounce Ux
                ux_sb = rtmp.tile([128, N], BF16, tag="ux_sb", name="ux_sb",
                                  bufs=2)
                nc.vector.tensor_copy(ux_sb[:], ptu["x"][:])
                nc.vector.tensor_mul(yv[:, cic, :], ux_sb[:], ptu["y"][:])
            return (s, yv, Ep, rscol)

        def emit_g(s):
            """g projections (fp8 DoubleRow, (N, CI) layout; col CI is the
            exact dithered residual colsum). Small PSUM pool, no deps on the
            previous pair's tail, so the PE enters each pair running."""
            j = s % 2
            x8_, y8_ = in_tiles[s]
            s8_ = {"x": x8_, "y": y8_}
            gT = {}
            rscol = rtmp.tile([128, MC], F32, tag=f"rscol{j}",
                              name="rscol", bufs=2)
            for st in ("x", "y"):
                w = w_sb[f"wt_g{st}"]
                dst = proj.tile([128, MC, CI], FP8, tag=f"gT{st}{j}",
                                name=f"gT{st}", bufs=2)
                gT[st] = dst
                for mc_ in range(MC):
                    pt = small()
                    has_b = st in gb_sb
                    for kp in range(KC // 2):
                        mmdr(pt[:, :CI + 1],
                             s8_[st][:, 2 * kp:2 * kp + 2,
                                     mc_ * 128:(mc_ + 1) * 128],
                             w[:, 2 * kp:2 * kp + 2, :CI + 1],
                             kp == 0, (kp == KC // 2 - 1) and not has_b)
                    if has_b:
                        mm(pt[:, :CI], ones_row[:], gb_sb[st][:], False, True)
                    nc.vector.tensor_copy(dst[:, mc_, :], pt[:, :CI])
                    if st == "x":
                        nc.vector.tensor_scalar_mul(
                            rscol[:, mc_:mc_ + 1], pt[:, CI:CI + 1], gs_sc)
                    else:
                        nc.vector.scalar_tensor_tensor(
                            rscol[:, mc_:mc_ + 1], pt[:, CI:CI + 1], go_sc,
                            rscol[:, mc_:mc_ + 1], ALU.mult, ALU.add)
            return gT, rscol

        def emit_proj(s, st, pj):
            """theta+phi projection block for one stream of sample s."""
            j = s % 2
            x8_, y8_ = in_tiles[s]
            s8_ = {"x": x8_, "y": y8_}
            for pr in ("t", "p"):
                w = w_sb[f"wt_{pr}{st}"]
                dst = proj.tile([128, CIC, N], FP8, tag=f"pj_{pr}{st}{j}",
                                name=f"pj_{pr}{st}")
                pj[pr + st] = dst
                for cic in range(CIC):
                    pt = med()
                    for kp in range(KC // 2):
                        for o, f in NH:
                            mmdr(pt[:, o:o + f],
                                 w[:, 2 * kp:2 * kp + 2,
                                   cic * 128:(cic + 1) * 128],
                                 s8_[st][:, 2 * kp:2 * kp + 2, o:o + f],
                                 kp == 0, kp == KC // 2 - 1)
                    if has_tpb:
                        nc.scalar.activation(
                            dst[:, cic, :], pt[:], AF.Identity,
                            bias=tb_sb[f"b_{pr}{st}"][:, cic:cic + 1])
                    else:
                        nc.scalar.copy(dst[:, cic, :], pt[:])

        def emit_logits(s, gstuff, fixb_item):
            """Logits + exp + S/Ep elementwise for sample s; the previous
            pair's fixup_b transposes slot in after mc 1."""
            j = s % 2
            pj = pj_state[j]
            E = attp.tile([128, MC, 2, N], BF16, tag=f"E{j}", name="E")
            S = attp.tile([128, MC, N], FP8E5, tag=f"S{j}", name="S", bufs=2)
            Ep = attp.tile([128, 2, 3, N], BF16, tag=f"Ep{j}", name="Ep",
                           bufs=2)
            for mc_ in range(MC):
                for sti, st in enumerate(("x", "y")):
                    pt = med()
                    for o, f in NH:
                        mmdr(pt[:, o:o + f],
                             pj["p" + st][:, :, mc_ * 128:(mc_ + 1) * 128],
                             pj["t" + st][:, :, o:o + f], True, True)
                    nc.scalar.activation(E[:, mc_, sti, :], pt[:],
                                         AF.Exp, bias=expb[:])
                # the first two S chunks go to GpSimd: slow, but consumed a
                # full phase later by the deferred apply
                eng = nc.gpsimd if mc_ < 2 else nc.vector
                eng.tensor_mul(S[:, mc_, :], E[:, mc_, 0, :],
                               E[:, mc_, 1, :])
                if mc_ % 2 == 1:
                    i = mc_ // 2
                    for sti in range(2):
                        nc.gpsimd.tensor_add(Ep[:, sti, i, :],
                                             E[:, 2 * i, sti, :],
                                             E[:, 2 * i + 1, sti, :])
                if mc_ == 1 and fixb_item is not None:
                    emit_fixup_b(fixb_item)
            gT, rscol = gstuff
            return (s, gT, S, Ep, rscol)

        # ---- paired schedule: one g-block per pair absorbs one exp tail;
        # deferred apply/fixups of the previous pair interleave between the
        # projection blocks ----
        pj_state = [{}, {}]
        pending = []   # apply_pending items from the previous pair
        fixes = []
        fixbs = []
        for r in range(BPC // 2):
            a, b = 2 * r, 2 * r + 1
            ga_ = emit_g(a)
            gb_ = emit_g(b)
            pj_state[0] = {}
            pj_state[1] = {}
            emit_proj(a, "x", pj_state[0])
            if pending:
                fixes.append(emit_apply(pending.pop(0)))
            emit_proj(b, "x", pj_state[1])
            if pending:
                fixes.append(emit_apply(pending.pop(0)))
            emit_proj(a, "y", pj_state[0])
            if fixes:
                fixbs.append(emit_fixup_a(fixes.pop(0)))
            for s2 in range(2 * r + 2, min(2 * r + 4, BPC)):
                in_tiles.append(load_inputs(s2))
            pa = emit_logits(a, ga_, fixbs.pop(0) if fixbs else None)
            emit_proj(b, "y", pj_state[1])
            if fixes:
                fixbs.append(emit_fixup_a(fixes.pop(0)))
            pb = emit_logits(b, gb_, fixbs.pop(0) if fixbs else None)
            pending = [pa, pb]

        # tail: interleave the last pair's applies with the Z/q fixup
        # pieces so the PE stays dense while the exp tails drain
        fa = emit_apply(pending[0])
        za = emit_fixup_z(fa)
        fxa = emit_fixup_q(za)
        fb = emit_apply(pending[1])
        emit_fixup_b(fxa)
        zb = emit_fixup_z(fb)
        emit_fixup_b(emit_fixup_q(zb))

        # ---- head ----
        hp = med()
        for j in range(MC):
            mm(hp[:BPC, :HOUT], pooledT[:, :, j], hwT[:, j, :],
               j == 0, (j == MC - 1) and not has_hb)
        if has_hb:
            mm(hp[:BPC, :HOUT], ones_row[:, :BPC], hb[:], False, True)
        out_sb = rows.tile([BPC, HOUT], F32, tag="out_sb", name="out_sb")
        nc.scalar.copy(out_sb[:], hp[:BPC, :HOUT])
        nc.sync.dma_start(d_out[:], out_sb[:])

    nc.compile()
    return nc


def _prepare(inputs):
    f = lambda k: np.ascontiguousarray(np.asarray(inputs[k], dtype=np.float32))
    bf = lambda a: np.ascontiguousarray(np.asarray(a, dtype=ml_dtypes.bfloat16))
    sar, opt = f("sar"), f("opt")
    ga = float(np.asarray(inputs["gamma_att"]).reshape(-1)[0])
    go = float(np.asarray(inputs["gamma_opt"]).reshape(-1)[0])
    gs = float(np.asarray(inputs["gamma_sar"]).reshape(-1)[0])
    W_w, W_b = f("W_w"), f("W_b")
    head_w, head_b = f("head_w"), f("head_b")

    wbar = (ga / C) * W_w.sum(axis=0)  # (CI,)
    bbar = (ga / C) * float(W_b.sum())
    # fold the pooled-constant through the head: out += bbar * head_w.sum(1)
    hb_eff = head_b + bbar * head_w.sum(axis=1)  # (HOUT,)

    gb_x, gb_y = f("g_sar_b"), f("g_opt_b")
    tpb = [f(k) for k in ("theta_sar_b", "phi_sar_b", "theta_opt_b",
                          "phi_opt_b")]
    has_gb_x = bool(np.any(gb_x))
    has_gb_y = bool(np.any(gb_y))
    has_hb = bool(np.any(hb_eff))
    has_tpb = bool(any(np.any(b) for b in tpb))

    key = (has_gb_x, has_gb_y, has_hb, has_tpb, gs / C, go / C)
    if key not in _cached:
        _cached[key] = _build(*key)
    nc = _cached[key]

    # pack inputs: (B, C, N) -> per-core (BPC, 128, KC*N) partition-major,
    # e4m3 with error diffusion along the channel dim
    def pack_in(a):
        d = _dither8(a, 1)  # (B, C, N) e4m3
        d = d.reshape(B, KC, 128, N).transpose(0, 2, 1, 3).reshape(B, 128, KC * N)
        return np.ascontiguousarray(d)

    sar_p, opt_p = pack_in(sar), pack_in(opt)

    def pack_gw(w, gbcol_unused=None):
        # (CI, C) -> wT (C, CI) + ones col -> padded (C, GW) -> (128, KC*GW)
        wt = np.concatenate(
            [w.T, np.ones((C, 1), np.float32),
             np.zeros((C, GW - CI - 1), np.float32)], axis=1)
        d = _dither8(wt, 0)
        return np.ascontiguousarray(
            d.reshape(KC, 128, GW).transpose(1, 0, 2).reshape(128, KC * GW))

    common = {
        "wt_tx": _pack8(f("theta_sar_w").T).reshape(128, KC * CI),
        "wt_px": _pack8(f("phi_sar_w").T).reshape(128, KC * CI),
        "wt_ty": _pack8(f("theta_opt_w").T).reshape(128, KC * CI),
        "wt_py": _pack8(f("phi_opt_w").T).reshape(128, KC * CI),
        "wt_gx": pack_gw(f("g_sar_w")),
        "wt_gy": pack_gw(f("g_opt_w")),
        "hwT": _packbf(head_w.T),
        "wbar": bf(wbar),
        "ones_col": np.ones((128, 1), ml_dtypes.bfloat16),
        "ident": np.eye(4, dtype=np.float32),
        "identr": np.ones((128, 1), np.float32),
        "expb": np.full((128, 1), EXP_SHIFT, np.float32),
    }
    if has_tpb:
        common.update({"b_tx": tpb[0], "b_px": tpb[1],
                       "b_ty": tpb[2], "b_py": tpb[3]})
    if has_gb_x or has_gb_y or has_hb:
        common["ones_row"] = np.ones((1, 128), ml_dtypes.bfloat16)
    if has_gb_x:
        common["gb_x"] = bf(gb_x.reshape(1, CI))
    if has_gb_y:
        common["gb_y"] = bf(gb_y.reshape(1, CI))
    if has_hb:
        common["hb"] = bf(hb_eff.reshape(1, HOUT))

    in_maps = []
    for c in range(NCORES):
        m = dict(common)
        m["sar8"] = np.ascontiguousarray(sar_p[c * BPC:(c + 1) * BPC])
        m["opt8"] = np.ascontiguousarray(opt_p[c * BPC:(c + 1) * BPC])
        in_maps.append(m)
    return nc, in_maps


def kernel(**inputs):
    nc, in_maps = _prepare(inputs)
    res = run_bass_kernel_spmd(nc, in_maps, core_ids=list(range(NCORES)))
    return np.concatenate([res.results[c]["out"] for c in range(NCORES)], axis=0)


if __name__ == "__main__":
    rng = np.random.default_rng(0)
    ins = {
        "sar": rng.standard_normal((B, C, N), dtype=np.float32),
        "opt": rng.standard_normal((B, C, N), dtype=np.float32),
    }
    for nm in ("g_sar", "g_opt", "theta_sar", "theta_opt", "phi_sar", "phi_opt"):
        ins[nm + "_w"] = 0.02 * rng.standard_normal((CI, C), dtype=np.float32)
        ins[nm + "_b"] = np.zeros((CI,), np.float32)
    ins["W_w"] = 0.02 * rng.standard_normal((C, CI), dtype=np.float32)
    ins["W_b"] = np.zeros((C,), np.float32)
    ins["head_w"] = 0.02 * rng.standard_normal((HOUT, N), dtype=np.float32)
    ins["head_b"] = np.zeros((HOUT,), np.float32)
    ins["gamma_sar"] = np.asarray([0.3], np.float32)
    ins["gamma_opt"] = np.asarray([1.0], np.float32)
    ins["gamma_att"] = np.asarray([1.0], np.float32)
    out = kernel(**ins)
    print(out.shape, out.dtype, np.abs(out).mean())


# revision 59
# speedup vs baseline: 1.0089x; 1.0089x over previous
"""Trainium2 Bass kernel for nn_CAFF_3100966388292.

Dual-stream (SAR/OPT) cross-attention fusion net:
  theta/phi/g 1x1-conv projections on both streams, per-sample NxN attention
  maps fused elementwise, both value streams attended, product taken, output
  1x1-conv + residual + channel-mean pool + linear head.

Strategy (pure data parallel, 4 samples per core on 8 cores):
  * All big matmuls in fp8 DoubleRow (theta/phi/g projections, logits,
    attention-apply). Inputs are host-quantized to e4m3 with error diffusion
    along the channel dim, which keeps channel-contractions and the residual
    colsum (ridden as a ones-column on the g weights) near-bf16 accurate.
  * Attention logits computed TRANSPOSED: L^T(m,n) = phi^T theta, so the
    contracted dim (m) of att@g lands on PSUM partitions naturally.
  * S = Ex*Ey stored e5m2 (global EXP_SHIFT=-8 per stream keeps all row
    maxima inside e5m2 range; e4m3 overflows); the attention-apply contracts
    S against e4m3 g tiles in DoubleRow, halving the baseline's biggest
    bf16 matmul.
  * Softmax denominators: one level of bf16 pair-adds over the six E chunks
    (on GpSimd, consumed a full phase later), then short ones-column
    matmuls; the (Zx*Zy)^-2 fixup runs COLUMNAR on [128,6] tiles after PE
    transposes of the row operands (the serial [1,768] DVE reciprocal of
    the baseline was ~5us per sample). The Zy row and the qraw matvec row
    share one PSUM tile at partition bases 0/64, so a single [65,768] ACT
    copy extracts both (ACT cost depends only on free size); the
    partition-64 transpose uses a partition-replicated identity +
    explicit tile_position.
  * The final W-projection + residual + channel-mean + head collapse
    algebraically:
      pooled(n) = R2(n)*qraw(n) + (ga/C)*sum(W_b) + rs(n),
      rs(n)     = (go/C)*colsum(opt)(n) + (gs/C)*colsum(sar)(n)
      qraw(n)   = sum_ci wbar(ci) * Ux(ci,n) * Uy(ci,n),
      wbar      = (ga/C) * W_w.sum(0)
    which removes the (C,CI)x(CI,N) W matmul entirely.
  * PSUM: a [128,768] "med" pool (bufs=3) + a 1-bank "small" pool (bufs=2)
    for the g-projections = 8 banks.
  * Samples are processed in PAIRS with all cross-engine chains deferred by
    one phase: each pair opens with BOTH samples' g-projections (small
    PSUM pool, no deps on the previous pair) which absorbs the previous
    pair's exp drain and keeps the PE HAM-warm; the previous pair's
    attention-applies and fixup chains interleave between this pair's
    theta/phi blocks; each sample's logits emit as early as possible.
    Engine FIFO order == emission order, so every deferred chain is placed
    where its dependencies are already drained (measured, not guessed:
    several "obvious" interleavings lose 10-25us to head-of-line blocking).
"""

import sys
import types

import ml_dtypes
import numpy as np

# The agent image's antenv package lacks axon_hooks; register the equivalent
# NTFF hook so run_bass_kernel_spmd(trace=True) works if ever requested.
try:  # pragma: no cover
    import antenv.axon_hooks  # noqa: F401
except ImportError:
    try:
        from trn_agent_boot.trn_boot import _ntff_profile_via_ctypes

        _hook = _ntff_profile_via_ctypes("/opt/axon/libaxon_pjrt.so")
        _mod = types.ModuleType("antenv.axon_hooks")
        _mod.get_axon_ntff_profile_hook = lambda: _hook
        _mod.set_axon_ntff_profile_hook = lambda h: None
        sys.modules["antenv.axon_hooks"] = _mod
    except Exception:
        pass

import concourse.bass as bass
import concourse.tile as tile
from concourse import bacc, mybir
from concourse.bass_utils import run_bass_kernel_spmd

F32 = mybir.dt.float32
BF16 = mybir.dt.bfloat16
FP8 = mybir.dt.float8e4
FP8E5 = mybir.dt.float8e5
EXP_SHIFT = -8.0  # per-stream logit shift; cancels exactly, keeps S in e5m2

B, C, CI, N, HOUT = 32, 512, 256, 768, 256
NCORES = 8
BPC = B // NCORES  # samples per core
KC = C // 128  # 4 k-chunks over channels
MC = N // 128  # 6 chunks over positions
CIC = CI // 128  # 2 chunks over inner channels
GW = 272  # fp8 g-weight free-dim padding (257 cols, DR needs step%16==0)
# matmul windows over N: DoubleRow streams at most 1024 moving elements
# (512 output cols), and a 512-col fp32 window fills one 2KB PSUM bank
NH = ((0, 512), (512, 256))

_cached = {}


def _dither8(a, axis):
    """e4m3 quantization with error diffusion along `axis` (preserves sums)."""
    a = np.moveaxis(np.asarray(a, dtype=np.float32), axis, 0)
    out = np.empty(a.shape, dtype=ml_dtypes.float8_e4m3fn)
    carry = np.zeros(a.shape[1:], np.float32)
    for i in range(a.shape[0]):
        v = a[i] + carry
        q = v.astype(ml_dtypes.float8_e4m3fn)
        carry = v - q.astype(np.float32)
        out[i] = q
    return np.moveaxis(out, 0, axis)


def _pack8(a):
    """(R, F) host array -> (128, R//128, F) partition-major, dithered e4m3."""
    a = np.asarray(a, dtype=np.float32)
    r, f = a.shape
    k = r // 128
    d = _dither8(a, 0)
    return np.ascontiguousarray(d.reshape(k, 128, f).transpose(1, 0, 2))


def _packbf(a):
    """(R, F) host array -> (128, R//128 * F) partition-major bf16."""
    a = np.asarray(a, dtype=np.float32)
    r, f = a.shape
    k = r // 128
    return np.ascontiguousarray(
        a.reshape(k, 128, f).transpose(1, 0, 2).reshape(128, k * f)
    ).astype(ml_dtypes.bfloat16)


def _build(has_gb_x, has_gb_y, has_hb, has_tpb, gs_sc, go_sc):
    nc = bacc.Bacc("TRN2", target_bir_lowering=False, debug=False)
    AF = mybir.ActivationFunctionType
    ALU = mybir.AluOpType

    def mm(out, lhsT, rhs, start, stop):
        nc.tensor.matmul(out, lhsT, rhs, start=start, stop=stop)

    def mmdr(out, lhsT, rhs, start, stop):
        nc.tensor.matmul(out, lhsT, rhs, start=start, stop=stop,
                         perf_mode=mybir.MatmulPerfMode.DoubleRow)

    # inputs host-packed to (BPC, 128, KC*N) partition-major dithered e4m3
    d_sar8 = nc.dram_tensor("sar8", [BPC, 128, KC * N], FP8, kind="ExternalInput")
    d_opt8 = nc.dram_tensor("opt8", [BPC, 128, KC * N], FP8, kind="ExternalInput")
    # host-pretransposed + packed projection weights
    d_w = {}
    for nm in ("wt_tx", "wt_px", "wt_ty", "wt_py"):
        d_w[nm] = nc.dram_tensor(nm, [128, KC * CI], FP8, kind="ExternalInput")
    for nm in ("wt_gx", "wt_gy"):  # g weights carry a ones column at 256
        d_w[nm] = nc.dram_tensor(nm, [128, KC * GW], FP8, kind="ExternalInput")
    d_hwT = nc.dram_tensor("hwT", [128, MC * HOUT], BF16, kind="ExternalInput")
    d_wbar = nc.dram_tensor("wbar", [CI], BF16, kind="ExternalInput")
    d_tb = {}
    if has_tpb:  # theta/phi per-partition bias columns (CI,), fp32 (ACT bias)
        for nm in ("b_tx", "b_px", "b_ty", "b_py"):
            d_tb[nm] = nc.dram_tensor(nm, [CI], F32, kind="ExternalInput")
    d_ones = nc.dram_tensor("ones_col", [128, 1], BF16, kind="ExternalInput")
    need_onesr = has_gb_x or has_gb_y or has_hb
    if need_onesr:
        d_onesr = nc.dram_tensor("ones_row", [1, 128], BF16, kind="ExternalInput")
    d_ident = nc.dram_tensor("ident", [4, 4], F32, kind="ExternalInput")
    d_identr = nc.dram_tensor("identr", [128, 1], F32, kind="ExternalInput")
    d_expb = nc.dram_tensor("expb", [128, 1], F32, kind="ExternalInput")
    d_gb = {}
    if has_gb_x:
        d_gb["x"] = nc.dram_tensor("gb_x", [1, CI], BF16, kind="ExternalInput")
    if has_gb_y:
        d_gb["y"] = nc.dram_tensor("gb_y", [1, CI], BF16, kind="ExternalInput")
    if has_hb:
        d_hb = nc.dram_tensor("hb", [1, HOUT], BF16, kind="ExternalInput")
    d_out = nc.dram_tensor("out", [BPC, HOUT], F32, kind="ExternalOutput")

    with tile.TileContext(nc) as tc, \
            tc.tile_pool(name="wts", bufs=1) as wts, \
            tc.tile_pool(name="inp", bufs=2) as inp, \
            tc.tile_pool(name="proj", bufs=1) as proj, \
            tc.tile_pool(name="att", bufs=1) as attp, \
            tc.tile_pool(name="rows", bufs=1) as rows, \
            tc.tile_pool(name="rtmp", bufs=4) as rtmp, \
            tc.tile_pool(name="psP", bufs=3, space="PSUM") as psP, \
            tc.tile_pool(name="psS", bufs=2, space="PSUM") as psS:

        def med():
            return psP.tile([128, N], F32, tag="ps", name="ps")

        def small():
            return psS.tile([128, 512], F32, tag="small", name="small")

        # ---- DMAs in strict first-use order: the queues are FIFO, so
        # everything emitted ahead of the first matmul's dependencies delays
        # kernel start ----
        def load_w(nm, cols=CI):
            t = wts.tile([128, KC, cols], FP8, tag=nm, name=nm)
            nc.sync.dma_start(t[:], d_w[nm].ap().rearrange("p (k f) -> p k f", k=KC))
            return t

        w_sb = {}
        # the first matmul is the sample-0 g-projection: its weight chunks
        # and the first input chunks gate the whole kernel, so issue those
        # pieces first
        t = wts.tile([128, KC, GW], FP8, tag="wt_gx", name="wt_gx")
        w_sb["wt_gx"] = t
        nc.sync.dma_start(t[:, 0:2, :],
                          d_w["wt_gx"].ap()[:, :2 * GW].rearrange(
                              "p (k f) -> p k f", k=2))
        x8_0 = inp.tile([128, KC, N], FP8, tag="x80", name="x8")
        nc.sync.dma_start(x8_0[:, 0:2, :],
                          d_sar8[0][:, :2 * N].rearrange("p (k n) -> p k n", k=2))
        nc.sync.dma_start(
            t[:, 2:, :],
            d_w["wt_gx"].ap()[:, 2 * GW:].rearrange("p (k f) -> p k f", k=KC - 2))
        nc.sync.dma_start(x8_0[:, 2:, :],
                          d_sar8[0][:, 2 * N:].rearrange("p (k n) -> p k n", k=2))
        w_sb["wt_gy"] = load_w("wt_gy", GW)
        y8_0 = inp.tile([128, KC, N], FP8, tag="y80", name="y8")
        nc.sync.dma_start(y8_0[:], d_opt8[0].rearrange("p (k n) -> p k n", k=KC))
        w_sb["wt_tx"] = load_w("wt_tx")
        w_sb["wt_px"] = load_w("wt_px")
        tb_sb = {}
        if has_tpb:
            for nm, d in d_tb.items():
                t = wts.tile([128, CIC], F32, tag=nm, name=nm)
                nc.sync.dma_start(t[:], d.ap().rearrange("(k p) -> p k", p=128))
                tb_sb[nm] = t
        w_sb["wt_ty"] = load_w("wt_ty")
        w_sb["wt_py"] = load_w("wt_py")

        def load_inputs(s):
            j = s % 2
            x8 = inp.tile([128, KC, N], FP8, tag=f"x8{j}", name="x8")
            y8 = inp.tile([128, KC, N], FP8, tag=f"y8{j}", name="y8")
            nc.sync.dma_start(x8[:], d_sar8[s].rearrange("p (k n) -> p k n", k=KC))
            nc.sync.dma_start(y8[:], d_opt8[s].rearrange("p (k n) -> p k n", k=KC))
            return x8, y8

        in_tiles = [(x8_0, y8_0)]
        in_tiles.append(load_inputs(1))

        # ---- small constants (all needed later than the projections) ----
        wbar = wts.tile([128, CIC], BF16, tag="wbar", name="wbar")
        nc.sync.dma_start(wbar[:], d_wbar.ap().rearrange("(k p) -> p k", p=128))
        ones_col = wts.tile([128, 1], BF16, tag="ones_col", name="ones_col")
        nc.sync.dma_start(ones_col[:], d_ones.ap())
        ident = wts.tile([4, 4], F32, tag="ident", name="ident")
        nc.sync.dma_start(ident[:], d_ident.ap())
        identr = wts.tile([128, 1], F32, tag="identr", name="identr")
        nc.sync.dma_start(identr[:], d_identr.ap())
        expb = wts.tile([128, 1], F32, tag="expb", name="expb")
        nc.sync.dma_start(expb[:], d_expb.ap())
        hwT = wts.tile([128, MC, HOUT], BF16, tag="hwT", name="hwT")
        nc.sync.dma_start(hwT[:], d_hwT.ap().rearrange("p (k f) -> p k f", k=MC))
        if need_onesr:
            ones_row = wts.tile([1, 128], BF16, tag="ones_row", name="ones_row")
            nc.sync.dma_start(ones_row[:], d_onesr.ap())
        gb_sb = {}
        for st, d in d_gb.items():
            t = wts.tile([1, CI], BF16, tag=f"gb_{st}", name=f"gb_{st}")
            nc.sync.dma_start(t[:], d.ap())
            gb_sb[st] = t
        if has_hb:
            hb = wts.tile([1, HOUT], BF16, tag="hb", name="hb")
            nc.sync.dma_start(hb[:], d_hb.ap())

        pooledT = rows.tile([128, BPC, MC], BF16, tag="pooledT", name="pooledT")

        def emit_fixup_z(fx):
            """Zx into PSUM tile A (partition 0); Zy into tile B partition
            0. The qraw matvec later lands in tile B at partition 64, so ONE
            ACT copy extracts zy+q together: ACT cost depends only on the
            free size, so a [65,768] copy costs the same as [1,768]."""
            s, yv, Ep, rscol = fx
            za_ps = med()
            zb_ps = med()
            for sti, zps in enumerate((za_ps, zb_ps)):
                for o, f in NH:
                    for i in range(3):
                        mm(zps[:1, o:o + f], ones_col[:],
                           Ep[:, sti, i, o:o + f], i == 0, i == 2)
            # copy zx out immediately: the PSUM slot is recycled soon after,
            # and a deferred reader would couple the PE queue to the fixup
            # chain (deadlock risk)
            zx = rtmp.tile([1, N], F32, tag="r_zx", name="zx", bufs=2)
            nc.scalar.copy(zx[:], za_ps[:1, :])
            return (s, yv, zb_ps, zx, rscol)

        def emit_fixup_q(fx):
            s, yv, zb_ps, zx, rscol = fx
            for cic in range(CIC):
                for o, f in NH:
                    mm(zb_ps[64:65, o:o + f], wbar[:, cic:cic + 1],
                       yv[:, cic, o:o + f], cic == 0, cic == CIC - 1)
            zqq = rtmp.tile([65, N], F32, tag="r_zq", name="zqq", bufs=2)
            nc.scalar.copy(zqq[:], zb_ps[0:65, :])
            zp = rtmp.tile([1, N], F32, tag="r_zp", name="zp", bufs=2)
            nc.vector.tensor_mul(zp[:], zx[:], zqq[0:1, :])
            return (s, zp, zqq, rscol)

        def emit_fixup_a(fx):
            return emit_fixup_q(emit_fixup_z(fx))

        def emit_fixup_b(fx):
            """12 PE transposes + columnar reciprocal chain (PE + DVE)."""
            s, zp, zqq, rscol = fx
            tr = small()
            for j in range(MC):
                nc.tensor.transpose(tr[:, j:j + 1],
                                    zp[:1, j * 128:(j + 1) * 128], ident[:1, :1])
            for j in range(MC):
                nc.tensor.transpose(tr[:, 8 + j:9 + j],
                                    zqq[64:65, j * 128:(j + 1) * 128],
                                    identr[64:65, :1], tile_position=(64, 0))
            r1 = rtmp.tile([128, MC], F32, tag="r_c1", name="r1", bufs=2)
            nc.vector.reciprocal(r1[:], tr[:, 0:MC])
            r2 = rtmp.tile([128, MC], F32, tag="r_c2", name="r2", bufs=2)
            nc.vector.tensor_mul(r2[:], r1[:], r1[:])
            p4 = rtmp.tile([128, MC], F32, tag="r_c3", name="p4", bufs=2)
            nc.vector.tensor_mul(p4[:], r2[:], tr[:, 8:8 + MC])
            nc.vector.tensor_add(pooledT[:, s, :], p4[:], rscol[:])

        def emit_apply(ap):
            """Attention-apply of sample s, emitted one projection phase
            later so the exp tail of sample s overlaps sample s+1's
            projection matmuls instead of stalling the PE."""
            s, gT, S, Ep, rscol = ap
            yv = attp.tile([128, CIC, N], BF16, tag=f"yv{s % 2}", name="yv",
                           bufs=2)
            for cic in range(CIC):
                ptu = {}
                for st in ("x", "y"):
                    pt = med()
                    ptu[st] = pt
                    for o, f in NH:
                        for jp in range(MC // 2):
                            mmdr(pt[:, o:o + f],
                                 gT[st][:, 2 * jp:2 * jp + 2,
                                        cic * 128:(cic + 1) * 128],
                                 S[:, 2 * jp:2 * jp + 2, o:o + f],
                                 jp == 0, jp == MC // 2 - 1)
                # DVE tensor_tensor cannot read two PSUM operands; bounce Ux
                ux_sb = rtmp.tile([128, N], BF16, tag="ux_sb", name="ux_sb",
                                  bufs=2)
                nc.vector.tensor_copy(ux_sb[:], ptu["x"][:])
                nc.vector.tensor_mul(yv[:, cic, :], ux_sb[:], ptu["y"][:])
            return (s, yv, Ep, rscol)

        def emit_g(s):
            """g projections (fp8 DoubleRow, (N, CI) layout; col CI is the
            exact dithered residual colsum). Small PSUM pool, no deps on the
            previous pair's tail, so the PE enters each pair running."""
            j = s % 2
            x8_, y8_ = in_tiles[s]
            s8_ = {"x": x8_, "y": y8_}
            gT = {}
            rscol = rtmp.tile([128, MC], F32, tag=f"rscol{j}",
                              name="rscol", bufs=2)
            for st in ("x", "y"):
                w = w_sb[f"wt_g{st}"]
                dst = proj.tile([128, MC, CI], FP8, tag=f"gT{st}{j}",
                                name=f"gT{st}", bufs=2)
                gT[st] = dst
                for mc_ in range(MC):
                    pt = small()
                    has_b = st in gb_sb
                    for kp in range(KC // 2):
                        mmdr(pt[:, :CI + 1],
                             s8_[st][:, 2 * kp:2 * kp + 2,
                                     mc_ * 128:(mc_ + 1) * 128],
                             w[:, 2 * kp:2 * kp + 2, :CI + 1],
                             kp == 0, (kp == KC // 2 - 1) and not has_b)
                    if has_b:
                        mm(pt[:, :CI], ones_row[:], gb_sb[st][:], False, True)
                    nc.vector.tensor_copy(dst[:, mc_, :], pt[:, :CI])
                    if st == "x":
                        nc.vector.tensor_scalar_mul(
                            rscol[:, mc_:mc_ + 1], pt[:, CI:CI + 1], gs_sc)
                    else:
                        nc.vector.scalar_tensor_tensor(
                            rscol[:, mc_:mc_ + 1], pt[:, CI:CI + 1], go_sc,
                            rscol[:, mc_:mc_ + 1], ALU.mult, ALU.add)
            return gT, rscol

        def emit_proj(s, st, pj):
            """theta+phi projection block for one stream of sample s."""
            j = s % 2
            x8_, y8_ = in_tiles[s]
            s8_ = {"x": x8_, "y": y8_}
            for pr in ("t", "p"):
                w = w_sb[f"wt_{pr}{st}"]
                dst = proj.tile([128, CIC, N], FP8, tag=f"pj_{pr}{st}{j}",
                                name=f"pj_{pr}{st}")
                pj[pr + st] = dst
                for cic in range(CIC):
                    pt = med()
                    for kp in range(KC // 2):
                        for o, f in NH:
                            mmdr(pt[:, o:o + f],
                                 w[:, 2 * kp:2 * kp + 2,
                                   cic * 128:(cic + 1) * 128],
                                 s8_[st][:, 2 * kp:2 * kp + 2, o:o + f],
                                 kp == 0, kp == KC // 2 - 1)
                    if has_tpb:
                        nc.scalar.activation(
                            dst[:, cic, :], pt[:], AF.Identity,
                            bias=tb_sb[f"b_{pr}{st}"][:, cic:cic + 1])
                    else:
                        nc.scalar.copy(dst[:, cic, :], pt[:])

        def emit_logits(s, gstuff, fixb_item):
            """Logits + exp + S/Ep elementwise for sample s; the previous
            pair's fixup_b transposes slot in after mc 1."""
            j = s % 2
            pj = pj_state[j]
            E = attp.tile([128, MC, 2, N], BF16, tag=f"E{j}", name="E")
            S = attp.tile([128, MC, N], FP8E5, tag=f"S{j}", name="S", bufs=2)
            Ep = attp.tile([128, 2, 3, N], BF16, tag=f"Ep{j}", name="Ep",
                           bufs=2)
            for mc_ in range(MC):
                for sti, st in enumerate(("x", "y")):
                    pt = med()
                    for o, f in NH:
                        mmdr(pt[:, o:o + f],
                             pj["p" + st][:, :, mc_ * 128:(mc_ + 1) * 128],
                             pj["t" + st][:, :, o:o + f], True, True)
                    nc.scalar.activation(E[:, mc_, sti, :], pt[:],
                                         AF.Exp, bias=expb[:])
                # the first two S chunks go to GpSimd: slow, but consumed a
                # full phase later by the deferred apply
                eng = nc.gpsimd if mc_ < 2 else nc.vector
                eng.tensor_mul(S[:, mc_, :], E[:, mc_, 0, :],
                               E[:, mc_, 1, :])
                if mc_ % 2 == 1:
                    i = mc_ // 2
                    for sti in range(2):
                        nc.gpsimd.tensor_add(Ep[:, sti, i, :],
                                             E[:, 2 * i, sti, :],
                                             E[:, 2 * i + 1, sti, :])
                if mc_ == 1 and fixb_item is not None:
                    emit_fixup_b(fixb_item)
            gT, rscol = gstuff
            return (s, gT, S, Ep, rscol)

        # ---- paired schedule: one g-block per pair absorbs one exp tail;
        # deferred apply/fixups of the previous pair interleave between the
        # projection blocks ----
        pj_state = [{}, {}]
        pending = []   # apply_pending items from the previous pair
        fixes = []
        fixbs = []
        for r in range(BPC // 2):
            a, b = 2 * r, 2 * r + 1
            ga_ = emit_g(a)
            gb_ = emit_g(b)
            pj_state[0] = {}
            pj_state[1] = {}
            emit_proj(a, "x", pj_state[0])
            if pending:
                fixes.append(emit_apply(pending.pop(0)))
            emit_proj(b, "x", pj_state[1])
            if pending:
                fixes.append(emit_apply(pending.pop(0)))
            emit_proj(a, "y", pj_state[0])
            if fixes:
                fixbs.append(emit_fixup_a(fixes.pop(0)))
            for s2 in range(2 * r + 2, min(2 * r + 4, BPC)):
                in_tiles.append(load_inputs(s2))
            pa = emit_logits(a, ga_, fixbs.pop(0) if fixbs else None)
            emit_proj(b, "y", pj_state[1])
            if fixes:
                fixbs.append(emit_fixup_a(fixes.pop(0)))
            pb = emit_logits(b, gb_, fixbs.pop(0) if fixbs else None)
            pending = [pa, pb]

        # tail: interleave the last pair's applies with the Z/q fixup
        # pieces so the PE stays dense while the exp tails drain
        fa = emit_apply(pending[0])
        za = emit_fixup_z(fa)
        fxa = emit_fixup_q(za)
        fb = emit_apply(pending[1])
        emit_fixup_b(fxa)
        zb = emit_fixup_z(fb)
        emit_fixup_b(emit_fixup_q(zb))

        # ---- head ----
        hp = med()
        for j in range(MC):
            mm(hp[:BPC, :HOUT], pooledT[:, :, j], hwT[:, j, :],
               j == 0, (j == MC - 1) and not has_hb)
        if has_hb:
            mm(hp[:BPC, :HOUT], ones_row[:, :BPC], hb[:], False, True)
        out_sb = rows.tile([BPC, HOUT], F32, tag="out_sb", name="out_sb")
        nc.scalar.copy(out_sb[:], hp[:BPC, :HOUT])
        nc.sync.dma_start(d_out[:], out_sb[:])

    nc.compile()
    return nc


def _prepare(inputs):
    f = lambda k: np.ascontiguousarray(np.asarray(inputs[k], dtype=np.float32))
    bf = lambda a: np.ascontiguousarray(np.asarray(a, dtype=ml_dtypes.bfloat16))
    sar, opt = f("sar"), f("opt")
    ga = float(np.asarray(inputs["gamma_att"]).reshape(-1)[0])
    go = float(np.asarray(inputs["gamma_opt"]).reshape(-1)[0])
    gs = float(np.asarray(inputs["gamma_sar"]).reshape(-1)[0])
    W_w, W_b = f("W_w"), f("W_b")
    head_w, head_b = f("head_w"), f("head_b")

    wbar = (ga / C) * W_w.sum(axis=0)  # (CI,)
    bbar = (ga / C) * float(W_b.sum())
    # fold the pooled-constant through the head: out += bbar * head_w.sum(1)
    hb_eff = head_b + bbar * head_w.sum(axis=1)  # (HOUT,)

    gb_x, gb_y = f("g_sar_b"), f("g_opt_b")
    tpb = [f(k) for k in ("theta_sar_b", "phi_sar_b", "theta_opt_b",
                          "phi_opt_b")]
    has_gb_x = bool(np.any(gb_x))
    has_gb_y = bool(np.any(gb_y))
    has_hb = bool(np.any(hb_eff))
    has_tpb = bool(any(np.any(b) for b in tpb))

    key = (has_gb_x, has_gb_y, has_hb, has_tpb, gs / C, go / C)
    if key not in _cached:
        _cached[key] = _build(*key)
    nc = _cached[key]

    # pack inputs: (B, C, N) -> per-core (BPC, 128, KC*N) partition-major,
    # e4m3 with error diffusion along the channel dim
    def pack_in(a):
        d = _dither8(a, 1)  # (B, C, N) e4m3
        d = d.reshape(B, KC, 128, N).transpose(0, 2, 1, 3).reshape(B, 128, KC * N)
        return np.ascontiguousarray(d)

    sar_p, opt_p = pack_in(sar), pack_in(opt)

    def pack_gw(w, gbcol_unused=None):
        # (CI, C) -> wT (C, CI) + ones col -> padded (C, GW) -> (128, KC*GW)
        wt = np.concatenate(
            [w.T, np.ones((C, 1), np.float32),
             np.zeros((C, GW - CI - 1), np.float32)], axis=1)
        d = _dither8(wt, 0)
        return np.ascontiguousarray(
            d.reshape(KC, 128, GW).transpose(1, 0, 2).reshape(128, KC * GW))

    common = {
        "wt_tx": _pack8(f("theta_sar_w").T).reshape(128, KC * CI),
        "wt_px": _pack8(f("phi_sar_w").T).reshape(128, KC * CI),
        "wt_ty": _pack8(f("theta_opt_w").T).reshape(128, KC * CI),
        "wt_py": _pack8(f("phi_opt_w").T).reshape(128, KC * CI),
        "wt_gx": pack_gw(f("g_sar_w")),
        "wt_gy": pack_gw(f("g_opt_w")),
        "hwT": _packbf(head_w.T),
        "wbar": bf(wbar),
        "ones_col": np.ones((128, 1), ml_dtypes.bfloat16),
        "ident": np.eye(4, dtype=np.float32),
        "identr": np.ones((128, 1), np.float32),
        "expb": np.full((128, 1), EXP_SHIFT, np.float32),
    }
    if has_tpb:
        common.update({"b_tx": tpb[0], "b_px": tpb[1],
                       "b_ty": tpb[2], "b_py": tpb[3]})
    if has_gb_x or has_gb_y or has_hb:
        common["ones_row"] = np.ones((1, 128), ml_dtypes.bfloat16)
    if has_gb_x:
        common["gb_x"] = bf(gb_x.reshape(1, CI))
    if has_gb_y:
        common["gb_y"] = bf(gb_y.reshape(1, CI))
    if has_hb:
        common["hb"] = bf(hb_eff.reshape(1, HOUT))

    in_maps = []
    for c in range(NCORES):
        m = dict(common)
        m["sar8"] = np.ascontiguousarray(sar_p[c * BPC:(c + 1) * BPC])
        m["opt8"] = np.ascontiguousarray(opt_p[c * BPC:(c + 1) * BPC])
        in_maps.append(m)
    return nc, in_maps


def kernel(**inputs):
    nc, in_maps = _prepare(inputs)
    res = run_bass_kernel_spmd(nc, in_maps, core_ids=list(range(NCORES)))
    return np.concatenate([res.results[c]["out"] for c in range(NCORES)], axis=0)


if __name__ == "__main__":
    rng = np.random.default_rng(0)
    ins = {
        "sar": rng.standard_normal((B, C, N), dtype=np.float32),
        "opt": rng.standard_normal((B, C, N), dtype=np.float32),
    }
    for nm in ("g_sar", "g_opt", "theta_sar", "theta_opt", "phi_sar", "phi_opt"):
        ins[nm + "_w"] = 0.02 * rng.standard_normal((CI, C), dtype=np.float32)
        ins[nm + "_b"] = np.zeros((CI,), np.float32)
    ins["W_w"] = 0.02 * rng.standard_normal((C, CI), dtype=np.float32)
    ins["W_b"] = np.zeros((C,), np.float32)
    ins["head_w"] = 0.02 * rng.standard_normal((HOUT, N), dtype=np.float32)
    ins["head_b"] = np.zeros((HOUT,), np.float32)
    ins["gamma_sar"] = np.asarray([0.3], np.float32)
    ins["gamma_opt"] = np.asarray([1.0], np.float32)
    ins["gamma_att"] = np.asarray([1.0], np.float32)
    out = kernel(**ins)
    print(out.shape, out.dtype, np.abs(out).mean())


# revision 60
# speedup vs baseline: 1.0246x; 1.0156x over previous
"""Trainium2 Bass kernel for nn_CAFF_3100966388292.

Dual-stream (SAR/OPT) cross-attention fusion net:
  theta/phi/g 1x1-conv projections on both streams, per-sample NxN attention
  maps fused elementwise, both value streams attended, product taken, output
  1x1-conv + residual + channel-mean pool + linear head.

Strategy (pure data parallel, 4 samples per core on 8 cores):
  * All big matmuls in fp8 DoubleRow (theta/phi/g projections, logits,
    attention-apply). Inputs are host-quantized to e4m3 with error diffusion
    along the channel dim, which keeps channel-contractions and the residual
    colsum (ridden as a ones-column on the g weights) near-bf16 accurate.
  * Attention logits computed TRANSPOSED: L^T(m,n) = phi^T theta, so the
    contracted dim (m) of att@g lands on PSUM partitions naturally.
  * S = Ex*Ey stored e5m2 (global EXP_SHIFT=-8 per stream keeps all row
    maxima inside e5m2 range; e4m3 overflows); the attention-apply contracts
    S against e4m3 g tiles in DoubleRow, halving the baseline's biggest
    bf16 matmul.
  * Softmax denominators: one level of bf16 pair-adds over the six E chunks
    (on GpSimd, consumed a full phase later), then short ones-column
    matmuls; the (Zx*Zy)^-2 fixup runs COLUMNAR on [128,6] tiles after PE
    transposes of the row operands (the serial [1,768] DVE reciprocal of
    the baseline was ~5us per sample). The Zy row and the qraw matvec row
    share one PSUM tile at partition bases 0/64, so a single [65,768] ACT
    copy extracts both (ACT cost depends only on free size); the
    partition-64 transpose uses a partition-replicated identity +
    explicit tile_position.
  * The final W-projection + residual + channel-mean + head collapse
    algebraically:
      pooled(n) = R2(n)*qraw(n) + (ga/C)*sum(W_b) + rs(n),
      rs(n)     = (go/C)*colsum(opt)(n) + (gs/C)*colsum(sar)(n)
      qraw(n)   = sum_ci wbar(ci) * Ux(ci,n) * Uy(ci,n),
      wbar      = (ga/C) * W_w.sum(0)
    which removes the (C,CI)x(CI,N) W matmul entirely.
  * PSUM: a [128,768] "med" pool (bufs=3) + a 1-bank "small" pool (bufs=2)
    for the g-projections = 8 banks.
  * Samples are processed in PAIRS with all cross-engine chains deferred by
    one phase: each pair opens with BOTH samples' g-projections (small
    PSUM pool, no deps on the previous pair) which absorbs the previous
    pair's exp drain and keeps the PE HAM-warm; the previous pair's
    attention-applies and fixup chains interleave between this pair's
    theta/phi blocks; each sample's logits emit as early as possible.
    Engine FIFO order == emission order, so every deferred chain is placed
    where its dependencies are already drained (measured, not guessed:
    several "obvious" interleavings lose 10-25us to head-of-line blocking).
"""

import sys
import types

import ml_dtypes
import numpy as np

# The agent image's antenv package lacks axon_hooks; register the equivalent
# NTFF hook so run_bass_kernel_spmd(trace=True) works if ever requested.
try:  # pragma: no cover
    import antenv.axon_hooks  # noqa: F401
except ImportError:
    try:
        from trn_agent_boot.trn_boot import _ntff_profile_via_ctypes

        _hook = _ntff_profile_via_ctypes("/opt/axon/libaxon_pjrt.so")
        _mod = types.ModuleType("antenv.axon_hooks")
        _mod.get_axon_ntff_profile_hook = lambda: _hook
        _mod.set_axon_ntff_profile_hook = lambda h: None
        sys.modules["antenv.axon_hooks"] = _mod
    except Exception:
        pass

import concourse.bass as bass
import concourse.tile as tile
from concourse import bacc, mybir
from concourse.bass_utils import run_bass_kernel_spmd

F32 = mybir.dt.float32
BF16 = mybir.dt.bfloat16
FP8 = mybir.dt.float8e4
FP8E5 = mybir.dt.float8e5
EXP_SHIFT = -8.0  # per-stream logit shift; cancels exactly, keeps S in e5m2

B, C, CI, N, HOUT = 32, 512, 256, 768, 256
NCORES = 8
BPC = B // NCORES  # samples per core
KC = C // 128  # 4 k-chunks over channels
MC = N // 128  # 6 chunks over positions
CIC = CI // 128  # 2 chunks over inner channels
GW = 272  # fp8 g-weight free-dim padding (257 cols, DR needs step%16==0)
# matmul windows over N: DoubleRow streams at most 1024 moving elements
# (512 output cols), and a 512-col fp32 window fills one 2KB PSUM bank
NH = ((0, 512), (512, 256))

_cached = {}


def _dither8(a, axis):
    """e4m3 quantization with error diffusion along `axis` (preserves sums)."""
    a = np.moveaxis(np.asarray(a, dtype=np.float32), axis, 0)
    out = np.empty(a.shape, dtype=ml_dtypes.float8_e4m3fn)
    carry = np.zeros(a.shape[1:], np.float32)
    for i in range(a.shape[0]):
        v = a[i] + carry
        q = v.astype(ml_dtypes.float8_e4m3fn)
        carry = v - q.astype(np.float32)
        out[i] = q
    return np.moveaxis(out, 0, axis)


def _pack8(a):
    """(R, F) host array -> (128, R//128, F) partition-major, dithered e4m3."""
    a = np.asarray(a, dtype=np.float32)
    r, f = a.shape
    k = r // 128
    d = _dither8(a, 0)
    return np.ascontiguousarray(d.reshape(k, 128, f).transpose(1, 0, 2))


def _packbf(a):
    """(R, F) host array -> (128, R//128 * F) partition-major bf16."""
    a = np.asarray(a, dtype=np.float32)
    r, f = a.shape
    k = r // 128
    return np.ascontiguousarray(
        a.reshape(k, 128, f).transpose(1, 0, 2).reshape(128, k * f)
    ).astype(ml_dtypes.bfloat16)


def _build(has_gb_x, has_gb_y, has_hb, has_tpb, gs_sc, go_sc):
    nc = bacc.Bacc("TRN2", target_bir_lowering=False, debug=False)
    AF = mybir.ActivationFunctionType
    ALU = mybir.AluOpType

    def mm(out, lhsT, rhs, start, stop):
        nc.tensor.matmul(out, lhsT, rhs, start=start, stop=stop)

    def mmdr(out, lhsT, rhs, start, stop):
        nc.tensor.matmul(out, lhsT, rhs, start=start, stop=stop,
                         perf_mode=mybir.MatmulPerfMode.DoubleRow)

    # inputs host-packed to (BPC, 128, KC*N) partition-major dithered e4m3
    d_sar8 = nc.dram_tensor("sar8", [BPC, 128, KC * N], FP8, kind="ExternalInput")
    d_opt8 = nc.dram_tensor("opt8", [BPC, 128, KC * N], FP8, kind="ExternalInput")
    # host-pretransposed + packed projection weights
    d_w = {}
    for nm in ("wt_tx", "wt_px", "wt_ty", "wt_py"):
        d_w[nm] = nc.dram_tensor(nm, [128, KC * CI], FP8, kind="ExternalInput")
    for nm in ("wt_gx", "wt_gy"):  # g weights carry a ones column at 256
        d_w[nm] = nc.dram_tensor(nm, [128, KC * GW], FP8, kind="ExternalInput")
    d_hwT = nc.dram_tensor("hwT", [128, MC * HOUT], BF16, kind="ExternalInput")
    d_wbar = nc.dram_tensor("wbar", [CI], BF16, kind="ExternalInput")
    d_tb = {}
    if has_tpb:  # theta/phi per-partition bias columns (CI,), fp32 (ACT bias)
        for nm in ("b_tx", "b_px", "b_ty", "b_py"):
            d_tb[nm] = nc.dram_tensor(nm, [CI], F32, kind="ExternalInput")
    d_ones = nc.dram_tensor("ones_col", [128, 1], BF16, kind="ExternalInput")
    need_onesr = has_gb_x or has_gb_y or has_hb
    if need_onesr:
        d_onesr = nc.dram_tensor("ones_row", [1, 128], BF16, kind="ExternalInput")
    d_ident = nc.dram_tensor("ident", [4, 4], F32, kind="ExternalInput")
    d_identr = nc.dram_tensor("identr", [128, 1], F32, kind="ExternalInput")
    d_expb = nc.dram_tensor("expb", [128, 1], F32, kind="ExternalInput")
    d_gb = {}
    if has_gb_x:
        d_gb["x"] = nc.dram_tensor("gb_x", [1, CI], BF16, kind="ExternalInput")
    if has_gb_y:
        d_gb["y"] = nc.dram_tensor("gb_y", [1, CI], BF16, kind="ExternalInput")
    if has_hb:
        d_hb = nc.dram_tensor("hb", [1, HOUT], BF16, kind="ExternalInput")
    d_out = nc.dram_tensor("out", [BPC, HOUT], F32, kind="ExternalOutput")

    with tile.TileContext(nc) as tc, \
            tc.tile_pool(name="wts", bufs=1) as wts, \
            tc.tile_pool(name="inp", bufs=2) as inp, \
            tc.tile_pool(name="proj", bufs=1) as proj, \
            tc.tile_pool(name="att", bufs=1) as attp, \
            tc.tile_pool(name="rows", bufs=1) as rows, \
            tc.tile_pool(name="rtmp", bufs=4) as rtmp, \
            tc.tile_pool(name="psP", bufs=3, space="PSUM") as psP, \
            tc.tile_pool(name="psS", bufs=2, space="PSUM") as psS:

        def med():
            return psP.tile([128, N], F32, tag="ps", name="ps")

        def small():
            return psS.tile([128, 512], F32, tag="small", name="small")

        # ---- DMAs in strict first-use order: the queues are FIFO, so
        # everything emitted ahead of the first matmul's dependencies delays
        # kernel start ----
        def load_w(nm, cols=CI):
            t = wts.tile([128, KC, cols], FP8, tag=nm, name=nm)
            nc.sync.dma_start(t[:], d_w[nm].ap().rearrange("p (k f) -> p k f", k=KC))
            return t

        w_sb = {}
        # the first matmul is the sample-0 g-projection: its weight chunks
        # and the first input chunks gate the whole kernel, so issue those
        # pieces first
        t = wts.tile([128, KC, GW], FP8, tag="wt_gx", name="wt_gx")
        w_sb["wt_gx"] = t
        nc.sync.dma_start(t[:, 0:2, :],
                          d_w["wt_gx"].ap()[:, :2 * GW].rearrange(
                              "p (k f) -> p k f", k=2))
        x8_0 = inp.tile([128, KC, N], FP8, tag="x80", name="x8")
        nc.sync.dma_start(x8_0[:, 0:2, :],
                          d_sar8[0][:, :2 * N].rearrange("p (k n) -> p k n", k=2))
        nc.sync.dma_start(
            t[:, 2:, :],
            d_w["wt_gx"].ap()[:, 2 * GW:].rearrange("p (k f) -> p k f", k=KC - 2))
        nc.sync.dma_start(x8_0[:, 2:, :],
                          d_sar8[0][:, 2 * N:].rearrange("p (k n) -> p k n", k=2))
        w_sb["wt_gy"] = load_w("wt_gy", GW)
        y8_0 = inp.tile([128, KC, N], FP8, tag="y80", name="y8")
        nc.sync.dma_start(y8_0[:], d_opt8[0].rearrange("p (k n) -> p k n", k=KC))
        w_sb["wt_tx"] = load_w("wt_tx")
        w_sb["wt_px"] = load_w("wt_px")
        tb_sb = {}
        if has_tpb:
            for nm, d in d_tb.items():
                t = wts.tile([128, CIC], F32, tag=nm, name=nm)
                nc.sync.dma_start(t[:], d.ap().rearrange("(k p) -> p k", p=128))
                tb_sb[nm] = t
        w_sb["wt_ty"] = load_w("wt_ty")
        w_sb["wt_py"] = load_w("wt_py")

        def load_inputs(s):
            j = s % 2
            x8 = inp.tile([128, KC, N], FP8, tag=f"x8{j}", name="x8")
            y8 = inp.tile([128, KC, N], FP8, tag=f"y8{j}", name="y8")
            nc.sync.dma_start(x8[:], d_sar8[s].rearrange("p (k n) -> p k n", k=KC))
            nc.sync.dma_start(y8[:], d_opt8[s].rearrange("p (k n) -> p k n", k=KC))
            return x8, y8

        in_tiles = [(x8_0, y8_0)]
        in_tiles.append(load_inputs(1))

        # ---- small constants (all needed later than the projections) ----
        wbar = wts.tile([128, CIC], BF16, tag="wbar", name="wbar")
        nc.sync.dma_start(wbar[:], d_wbar.ap().rearrange("(k p) -> p k", p=128))
        ones_col = wts.tile([128, 1], BF16, tag="ones_col", name="ones_col")
        nc.sync.dma_start(ones_col[:], d_ones.ap())
        ident = wts.tile([4, 4], F32, tag="ident", name="ident")
        nc.sync.dma_start(ident[:], d_ident.ap())
        identr = wts.tile([128, 1], F32, tag="identr", name="identr")
        nc.sync.dma_start(identr[:], d_identr.ap())
        expb = wts.tile([128, 1], F32, tag="expb", name="expb")
        nc.sync.dma_start(expb[:], d_expb.ap())
        hwT = wts.tile([128, MC, HOUT], BF16, tag="hwT", name="hwT")
        nc.sync.dma_start(hwT[:], d_hwT.ap().rearrange("p (k f) -> p k f", k=MC))
        if need_onesr:
            ones_row = wts.tile([1, 128], BF16, tag="ones_row", name="ones_row")
            nc.sync.dma_start(ones_row[:], d_onesr.ap())
        gb_sb = {}
        for st, d in d_gb.items():
            t = wts.tile([1, CI], BF16, tag=f"gb_{st}", name=f"gb_{st}")
            nc.sync.dma_start(t[:], d.ap())
            gb_sb[st] = t
        if has_hb:
            hb = wts.tile([1, HOUT], BF16, tag="hb", name="hb")
            nc.sync.dma_start(hb[:], d_hb.ap())

        pooledT = rows.tile([128, BPC, MC], BF16, tag="pooledT", name="pooledT")

        def emit_fixup_z(fx):
            """Zx into PSUM tile A (partition 0); Zy into tile B partition
            0. The qraw matvec later lands in tile B at partition 64, so ONE
            ACT copy extracts zy+q together: ACT cost depends only on the
            free size, so a [65,768] copy costs the same as [1,768]."""
            s, yv, Ep, rscol = fx
            za_ps = med()
            zb_ps = med()
            for sti, zps in enumerate((za_ps, zb_ps)):
                for o, f in NH:
                    for i in range(3):
                        mm(zps[:1, o:o + f], ones_col[:],
                           Ep[:, sti, i, o:o + f], i == 0, i == 2)
            # copy zx out immediately: the PSUM slot is recycled soon after,
            # and a deferred reader would couple the PE queue to the fixup
            # chain (deadlock risk)
            zx = rtmp.tile([1, N], F32, tag="r_zx", name="zx", bufs=2)
            nc.scalar.copy(zx[:], za_ps[:1, :])
            return (s, yv, zb_ps, zx, rscol)

        def emit_fixup_q(fx):
            s, yv, zb_ps, zx, rscol = fx
            for cic in range(CIC):
                for o, f in NH:
                    mm(zb_ps[64:65, o:o + f], wbar[:, cic:cic + 1],
                       yv[:, cic, o:o + f], cic == 0, cic == CIC - 1)
            zqq = rtmp.tile([65, N], F32, tag="r_zq", name="zqq", bufs=2)
            nc.scalar.copy(zqq[:], zb_ps[0:65, :])
            zp = rtmp.tile([1, N], F32, tag="r_zp", name="zp", bufs=2)
            nc.vector.tensor_mul(zp[:], zx[:], zqq[0:1, :])
            return (s, zp, zqq, rscol)

        def emit_fixup_a(fx):
            return emit_fixup_q(emit_fixup_z(fx))

        def emit_fixup_b(fx):
            """12 PE transposes + columnar reciprocal chain (PE + DVE)."""
            s, zp, zqq, rscol = fx
            tr = small()
            for j in range(MC):
                nc.tensor.transpose(tr[:, j:j + 1],
                                    zp[:1, j * 128:(j + 1) * 128], ident[:1, :1])
            for j in range(MC):
                nc.tensor.transpose(tr[:, 8 + j:9 + j],
                                    zqq[64:65, j * 128:(j + 1) * 128],
                                    identr[64:65, :1], tile_position=(64, 0))
            r1 = rtmp.tile([128, MC], F32, tag="r_c1", name="r1", bufs=2)
            nc.vector.reciprocal(r1[:], tr[:, 0:MC])
            r2 = rtmp.tile([128, MC], F32, tag="r_c2", name="r2", bufs=2)
            nc.vector.tensor_mul(r2[:], r1[:], r1[:])
            p4 = rtmp.tile([128, MC], F32, tag="r_c3", name="p4", bufs=2)
            nc.vector.tensor_mul(p4[:], r2[:], tr[:, 8:8 + MC])
            nc.vector.tensor_add(pooledT[:, s, :], p4[:], rscol[:])

        def emit_apply(ap):
            """Attention-apply of sample s, emitted one projection phase
            later so the exp tail of sample s overlaps sample s+1's
            projection matmuls instead of stalling the PE."""
            s, gT, S, Ep, rscol = ap
            yv = attp.tile([128, CIC, N], BF16, tag=f"yv{s % 2}", name="yv",
                           bufs=2)
            for cic in range(CIC):
                ptu = {}
                for st in ("x", "y"):
                    pt = med()
                    ptu[st] = pt
                    for o, f in NH:
                        for jp in range(MC // 2):
                            mmdr(pt[:, o:o + f],
                                 gT[st][:, 2 * jp:2 * jp + 2,
                                        cic * 128:(cic + 1) * 128],
                                 S[:, 2 * jp:2 * jp + 2, o:o + f],
                                 jp == 0, jp == MC // 2 - 1)
                # DVE tensor_tensor cannot read two PSUM operands; bounce Ux
                ux_sb = rtmp.tile([128, N], BF16, tag="ux_sb", name="ux_sb",
                                  bufs=2)
                nc.scalar.copy(ux_sb[:], ptu["x"][:])
                nc.vector.tensor_mul(yv[:, cic, :], ux_sb[:], ptu["y"][:])
            return (s, yv, Ep, rscol)

        def emit_g(s):
            """g projections (fp8 DoubleRow, (N, CI) layout; col CI is the
            exact dithered residual colsum). Small PSUM pool, no deps on the
            previous pair's tail, so the PE enters each pair running."""
            j = s % 2
            x8_, y8_ = in_tiles[s]
            s8_ = {"x": x8_, "y": y8_}
            gT = {}
            rscol = rtmp.tile([128, MC], F32, tag=f"rscol{j}",
                              name="rscol", bufs=2)
            for st in ("x", "y"):
                w = w_sb[f"wt_g{st}"]
                dst = proj.tile([128, MC, CI], FP8, tag=f"gT{st}{j}",
                                name=f"gT{st}", bufs=2)
                gT[st] = dst
                for mc_ in range(MC):
                    pt = small()
                    has_b = st in gb_sb
                    for kp in range(KC // 2):
                        mmdr(pt[:, :CI + 1],
                             s8_[st][:, 2 * kp:2 * kp + 2,
                                     mc_ * 128:(mc_ + 1) * 128],
                             w[:, 2 * kp:2 * kp + 2, :CI + 1],
                             kp == 0, (kp == KC // 2 - 1) and not has_b)
                    if has_b:
                        mm(pt[:, :CI], ones_row[:], gb_sb[st][:], False, True)
                    nc.vector.tensor_copy(dst[:, mc_, :], pt[:, :CI])
                    if st == "x":
                        nc.vector.tensor_scalar_mul(
                            rscol[:, mc_:mc_ + 1], pt[:, CI:CI + 1], gs_sc)
                    else:
                        nc.vector.scalar_tensor_tensor(
                            rscol[:, mc_:mc_ + 1], pt[:, CI:CI + 1], go_sc,
                            rscol[:, mc_:mc_ + 1], ALU.mult, ALU.add)
            return gT, rscol

        def emit_proj(s, st, pj):
            """theta+phi projection block for one stream of sample s."""
            j = s % 2
            x8_, y8_ = in_tiles[s]
            s8_ = {"x": x8_, "y": y8_}
            for pr in ("t", "p"):
                w = w_sb[f"wt_{pr}{st}"]
                dst = proj.tile([128, CIC, N], FP8, tag=f"pj_{pr}{st}{j}",
                                name=f"pj_{pr}{st}")
                pj[pr + st] = dst
                for cic in range(CIC):
                    pt = med()
                    for kp in range(KC // 2):
                        for o, f in NH:
                            mmdr(pt[:, o:o + f],
                                 w[:, 2 * kp:2 * kp + 2,
                                   cic * 128:(cic + 1) * 128],
                                 s8_[st][:, 2 * kp:2 * kp + 2, o:o + f],
                                 kp == 0, kp == KC // 2 - 1)
                    if has_tpb:
                        nc.scalar.activation(
                            dst[:, cic, :], pt[:], AF.Identity,
                            bias=tb_sb[f"b_{pr}{st}"][:, cic:cic + 1])
                    else:
                        nc.scalar.copy(dst[:, cic, :], pt[:])

        def emit_logits(s, gstuff, fixb_item):
            """Logits + exp + S/Ep elementwise for sample s; the previous
            pair's fixup_b transposes slot in after mc 1."""
            j = s % 2
            pj = pj_state[j]
            E = attp.tile([128, MC, 2, N], BF16, tag=f"E{j}", name="E")
            S = attp.tile([128, MC, N], FP8E5, tag=f"S{j}", name="S", bufs=2)
            Ep = attp.tile([128, 2, 3, N], BF16, tag=f"Ep{j}", name="Ep",
                           bufs=2)
            for mc_ in range(MC):
                for sti, st in enumerate(("x", "y")):
                    pt = med()
                    for o, f in NH:
                        mmdr(pt[:, o:o + f],
                             pj["p" + st][:, :, mc_ * 128:(mc_ + 1) * 128],
                             pj["t" + st][:, :, o:o + f], True, True)
                    nc.scalar.activation(E[:, mc_, sti, :], pt[:],
                                         AF.Exp, bias=expb[:])
                # the first two S chunks go to GpSimd: slow, but consumed a
                # full phase later by the deferred apply
                eng = nc.gpsimd if mc_ < 2 else nc.vector
                eng.tensor_mul(S[:, mc_, :], E[:, mc_, 0, :],
                               E[:, mc_, 1, :])
                if mc_ % 2 == 1:
                    i = mc_ // 2
                    for sti in range(2):
                        nc.gpsimd.tensor_add(Ep[:, sti, i, :],
                                             E[:, 2 * i, sti, :],
                                             E[:, 2 * i + 1, sti, :])
                if mc_ == 1 and fixb_item is not None:
                    emit_fixup_b(fixb_item)
            gT, rscol = gstuff
            return (s, gT, S, Ep, rscol)

        # ---- paired schedule: one g-block per pair absorbs one exp tail;
        # deferred apply/fixups of the previous pair interleave between the
        # projection blocks ----
        pj_state = [{}, {}]
        pending = []   # apply_pending items from the previous pair
        fixes = []
        fixbs = []
        for r in range(BPC // 2):
            a, b = 2 * r, 2 * r + 1
            ga_ = emit_g(a)
            gb_ = emit_g(b)
            pj_state[0] = {}
            pj_state[1] = {}
            emit_proj(a, "x", pj_state[0])
            if pending:
                fixes.append(emit_apply(pending.pop(0)))
            emit_proj(b, "x", pj_state[1])
            if pending:
                fixes.append(emit_apply(pending.pop(0)))
            emit_proj(a, "y", pj_state[0])
            if fixes:
                fixbs.append(emit_fixup_a(fixes.pop(0)))
            for s2 in range(2 * r + 2, min(2 * r + 4, BPC)):
                in_tiles.append(load_inputs(s2))
            pa = emit_logits(a, ga_, fixbs.pop(0) if fixbs else None)
            emit_proj(b, "y", pj_state[1])
            if fixes:
                fixbs.append(emit_fixup_a(fixes.pop(0)))
            pb = emit_logits(b, gb_, fixbs.pop(0) if fixbs else None)
            pending = [pa, pb]

        # tail: interleave the last pair's applies with the Z/q fixup
        # pieces so the PE stays dense while the exp tails drain
        fa = emit_apply(pending[0])
        za = emit_fixup_z(fa)
        fxa = emit_fixup_q(za)
        fb = emit_apply(pending[1])
        emit_fixup_b(fxa)
        zb = emit_fixup_z(fb)
        emit_fixup_b(emit_fixup_q(zb))

        # ---- head ----
        hp = med()
        for j in range(MC):
            mm(hp[:BPC, :HOUT], pooledT[:, :, j], hwT[:, j, :],
               j == 0, (j == MC - 1) and not has_hb)
        if has_hb:
            mm(hp[:BPC, :HOUT], ones_row[:, :BPC], hb[:], False, True)
        out_sb = rows.tile([BPC, HOUT], F32, tag="out_sb", name="out_sb")
        nc.scalar.copy(out_sb[:], hp[:BPC, :HOUT])
        nc.sync.dma_start(d_out[:], out_sb[:])

    nc.compile()
    return nc


def _prepare(inputs):
    f = lambda k: np.ascontiguousarray(np.asarray(inputs[k], dtype=np.float32))
    bf = lambda a: np.ascontiguousarray(np.asarray(a, dtype=ml_dtypes.bfloat16))
    sar, opt = f("sar"), f("opt")
    ga = float(np.asarray(inputs["gamma_att"]).reshape(-1)[0])
    go = float(np.asarray(inputs["gamma_opt"]).reshape(-1)[0])
    gs = float(np.asarray(inputs["gamma_sar"]).reshape(-1)[0])
    W_w, W_b = f("W_w"), f("W_b")
    head_w, head_b = f("head_w"), f("head_b")

    wbar = (ga / C) * W_w.sum(axis=0)  # (CI,)
    bbar = (ga / C) * float(W_b.sum())
    # fold the pooled-constant through the head: out += bbar * head_w.sum(1)
    hb_eff = head_b + bbar * head_w.sum(axis=1)  # (HOUT,)

    gb_x, gb_y = f("g_sar_b"), f("g_opt_b")
    tpb = [f(k) for k in ("theta_sar_b", "phi_sar_b", "theta_opt_b",
                          "phi_opt_b")]
    has_gb_x = bool(np.any(gb_x))
    has_gb_y = bool(np.any(gb_y))
    has_hb = bool(np.any(hb_eff))
    has_tpb = bool(any(np.any(b) for b in tpb))

    key = (has_gb_x, has_gb_y, has_hb, has_tpb, gs / C, go / C)
    if key not in _cached:
        _cached[key] = _build(*key)
    nc = _cached[key]

    # pack inputs: (B, C, N) -> per-core (BPC, 128, KC*N) partition-major,
    # e4m3 with error diffusion along the channel dim
    def pack_in(a):
        d = _dither8(a, 1)  # (B, C, N) e4m3
        d = d.reshape(B, KC, 128, N).transpose(0, 2, 1, 3).reshape(B, 128, KC * N)
        return np.ascontiguousarray(d)

    sar_p, opt_p = pack_in(sar), pack_in(opt)

    def pack_gw(w, gbcol_unused=None):
        # (CI, C) -> wT (C, CI) + ones col -> padded (C, GW) -> (128, KC*GW)
        wt = np.concatenate(
            [w.T, np.ones((C, 1), np.float32),
             np.zeros((C, GW - CI - 1), np.float32)], axis=1)
        d = _dither8(wt, 0)
        return np.ascontiguousarray(
            d.reshape(KC, 128, GW).transpose(1, 0, 2).reshape(128, KC * GW))

    common = {
        "wt_tx": _pack8(f("theta_sar_w").T).reshape(128, KC * CI),
        "wt_px": _pack8(f("phi_sar_w").T).reshape(128, KC * CI),
        "wt_ty": _pack8(f("theta_opt_w").T).reshape(128, KC * CI),
        "wt_py": _pack8(f("phi_opt_w").T).reshape(128, KC * CI),
        "wt_gx": pack_gw(f("g_sar_w")),
        "wt_gy": pack_gw(f("g_opt_w")),
        "hwT": _packbf(head_w.T),
        "wbar": bf(wbar),
        "ones_col": np.ones((128, 1), ml_dtypes.bfloat16),
        "ident": np.eye(4, dtype=np.float32),
        "identr": np.ones((128, 1), np.float32),
        "expb": np.full((128, 1), EXP_SHIFT, np.float32),
    }
    if has_tpb:
        common.update({"b_tx": tpb[0], "b_px": tpb[1],
                       "b_ty": tpb[2], "b_py": tpb[3]})
    if has_gb_x or has_gb_y or has_hb:
        common["ones_row"] = np.ones((1, 128), ml_dtypes.bfloat16)
    if has_gb_x:
        common["gb_x"] = bf(gb_x.reshape(1, CI))
    if has_gb_y:
        common["gb_y"] = bf(gb_y.reshape(1, CI))
    if has_hb:
        common["hb"] = bf(hb_eff.reshape(1, HOUT))

    in_maps = []
    for c in range(NCORES):
        m = dict(common)
        m["sar8"] = np.ascontiguousarray(sar_p[c * BPC:(c + 1) * BPC])
        m["opt8"] = np.ascontiguousarray(opt_p[c * BPC:(c + 1) * BPC])
        in_maps.append(m)
    return nc, in_maps


def kernel(**inputs):
    nc, in_maps = _prepare(inputs)
    res = run_bass_kernel_spmd(nc, in_maps, core_ids=list(range(NCORES)))
    return np.concatenate([res.results[c]["out"] for c in range(NCORES)], axis=0)


if __name__ == "__main__":
    rng = np.random.default_rng(0)
    ins = {
        "sar": rng.standard_normal((B, C, N), dtype=np.float32),
        "opt": rng.standard_normal((B, C, N), dtype=np.float32),
    }
    for nm in ("g_sar", "g_opt", "theta_sar", "theta_opt", "phi_sar", "phi_opt"):
        ins[nm + "_w"] = 0.02 * rng.standard_normal((CI, C), dtype=np.float32)
        ins[nm + "_b"] = np.zeros((CI,), np.float32)
    ins["W_w"] = 0.02 * rng.standard_normal((C, CI), dtype=np.float32)
    ins["W_b"] = np.zeros((C,), np.float32)
    ins["head_w"] = 0.02 * rng.standard_normal((HOUT, N), dtype=np.float32)
    ins["head_b"] = np.zeros((HOUT,), np.float32)
    ins["gamma_sar"] = np.asarray([0.3], np.float32)
    ins["gamma_opt"] = np.asarray([1.0], np.float32)
    ins["gamma_att"] = np.asarray([1.0], np.float32)
    out = kernel(**ins)
    print(out.shape, out.dtype, np.abs(out).mean())
